# revision 17
# baseline (speedup 1.0000x reference)
"""Trainium2 Bass kernel for a 16-head causal decoder block.

Sharding: 8 cores = 4 batches x 2 head-groups (8 heads each).
Per core: LN1 -> Q/K/V proj (its heads) -> causal attention -> partial Wo
-> pairwise ReduceScatter of x2 partials (chunked per 512-token qtile)
-> token-sharded LN2 + FFN (1024 tokens per core) -> output slice.

All matmuls run as float32r (FP22 truncation, full PE rate at N>=256).
LayerNorm affines are folded into the projection weights on the host.
"""

import os
import sys

import numpy as np


def _ensure_path():
    try:
        import concourse.bass  # noqa: F401
        return
    except ImportError:
        pass
    for p in ("/opt/trn_rl_repo", "/root/.axon_site/_ro/trn_rl_repo"):
        if os.path.isdir(p) and p not in sys.path:
            sys.path.insert(0, p)
    import concourse.bass  # noqa: F401


_ensure_path()

import concourse.bass as bass  # noqa: E402
import concourse.mybir as mybir  # noqa: E402
import concourse.tile as tile  # noqa: E402
from concourse import bacc  # noqa: E402
from concourse.masks import make_identity  # noqa: E402

F32 = mybir.dt.float32
FR = mybir.dt.float32r
AF = mybir.ActivationFunctionType
OP = mybir.AluOpType

DH = 64          # head dim
HLOC = 8         # heads per core
QTILE = 512      # query tile
EPS = 1e-5


def _fr(ap):
    return ap.bitcast(FR)


def build_decoder_nc(TB=2048, C=1024, FF=4096, n_devices=8, mock_rs=False):
    """Build the SPMD Bass program. TB: tokens/batch, C: embed, FF: ff dim."""
    assert TB % QTILE == 0 and C % 512 == 0 and FF % 128 == 0
    CC = C // 128          # C chunks
    NQT = TB // QTILE      # query tiles
    NKC = TB // 128        # key chunks
    NFC = FF // 128        # ff chunks
    NCT = C // 512         # output C tiles
    TOK = TB // 2          # output tokens per core
    HALF = TOK // 2        # ffn token half
    NHK = HALF // 128      # token chunks per ffn half
    NPAIR = HLOC // 2
    NQUAD = HLOC // 4
    NHC = HLOC * DH // 128  # attn-out hd chunks (4)
    groups = [[2 * i, 2 * i + 1] for i in range(n_devices // 2)]

    nc = bacc.Bacc("TRN2", target_bir_lowering=False, debug=False,
                   num_devices=n_devices)

    x_in = nc.dram_tensor("x", [TB, C], F32, kind="ExternalInput").ap()
    xh_in = nc.dram_tensor("xh", [TOK, C], F32, kind="ExternalInput").ap()
    wq_in = nc.dram_tensor("wq", [C, HLOC * DH], F32, kind="ExternalInput").ap()
    wk_in = nc.dram_tensor("wk", [C, HLOC * DH], F32, kind="ExternalInput").ap()
    wv_in = nc.dram_tensor("wv", [C, HLOC * DH], F32, kind="ExternalInput").ap()
    qb_in = nc.dram_tensor("qb", [128, NPAIR], F32, kind="ExternalInput").ap()
    kb_in = nc.dram_tensor("kb", [128, NPAIR], F32, kind="ExternalInput").ap()
    vb_in = nc.dram_tensor("vb", [HLOC * DH], F32, kind="ExternalInput").ap()
    wo_in = nc.dram_tensor("wo", [HLOC * DH, C], F32, kind="ExternalInput").ap()
    bo_in = nc.dram_tensor("bo", [C], F32, kind="ExternalInput").ap()
    w1_in = nc.dram_tensor("w1", [C, FF], F32, kind="ExternalInput").ap()
    b1_in = nc.dram_tensor("b1f", [128, NFC], F32, kind="ExternalInput").ap()
    w2_in = nc.dram_tensor("w2", [FF, C], F32, kind="ExternalInput").ap()
    b2_in = nc.dram_tensor("b2", [C], F32, kind="ExternalInput").ap()
    out_d = nc.dram_tensor("out", [TOK, C], F32, kind="ExternalOutput").ap()

    def bcast(ap1d, p=128):
        return bass.AP(tensor=ap1d.tensor, offset=ap1d.offset,
                       ap=[[0, p]] + [list(d) for d in ap1d.ap])

    n_sub = max(1, C // 512)
    sub = C // n_sub

    with tile.TileContext(nc) as tc:
        import contextlib
        with contextlib.ExitStack() as top:
            const = top.enter_context(tc.tile_pool(name="const", bufs=1))
            ident = const.tile([128, 128], F32)
            make_identity(nc, ident)
            ones1 = const.tile([1, DH], FR)
            ones1f = const.tile([1, DH], F32)
            nc.vector.memset(ones1f, 1.0)
            nc.scalar.copy(ones1, ones1f)
            eps_t = const.tile([128, 1], F32)
            nc.vector.memset(eps_t, EPS)

            attn_scope = contextlib.ExitStack()
            acts = attn_scope.enter_context(tc.tile_pool(name="acts", bufs=1))
            QT = acts.tile([128, NPAIR, TB], FR)    # Q^T, head pairs
            KT = acts.tile([128, NPAIR, TB], FR)    # K^T, head pairs
            V4 = acts.tile([128, NKC, HLOC, DH + 1], FR)  # V + ones col
            attnT = acts.tile([128, NHC, TB], FR)   # normalized attn out ^T
            onesv = const.tile([128, NKC * HLOC], F32)
            nc.vector.memset(onesv, 1.0)
            nc.vector.tensor_copy(
                out=V4[:, :, :, DH],
                in_=onesv.rearrange("p (a b) -> p a b", a=NKC))

            # ---------------- phase 1+2: LN1 + projections ----------------
            with contextlib.ExitStack() as s:
                cst1 = s.enter_context(tc.tile_pool(name="cst1", bufs=1))
                qb_sb = cst1.tile([128, NPAIR], F32)
                nc.sync.dma_start(qb_sb, qb_in)
                kb_sb = cst1.tile([128, NPAIR], F32)
                nc.sync.dma_start(kb_sb, kb_in)
                vb_bc = cst1.tile([128, HLOC * DH], F32)
                nc.sync.dma_start(vb_bc, bcast(vb_in))
                wqs = s.enter_context(tc.tile_pool(name="wqs", bufs=10))
                lnp = s.enter_context(tc.tile_pool(name="lnp", bufs=5))
                stp = s.enter_context(tc.tile_pool(name="stp", bufs=6))
                xntp = s.enter_context(tc.tile_pool(name="xntp", bufs=1))
                ps_t = s.enter_context(
                    tc.tile_pool(name="ps_t", bufs=3, space="PSUM"))
                ps_p = s.enter_context(
                    tc.tile_pool(name="ps_p", bufs=3, space="PSUM"))
                wq_r = wq_in.rearrange("(cc p) n -> p cc n", p=128)
                wk_r = wk_in.rearrange("(cc p) n -> p cc n", p=128)
                wv_r = wv_in.rearrange("(cc p) n -> p cc n", p=128)

                for ha in range(4):
                    hTB = TB // 4
                    PT = min(512, hTB)
                    xnT = xntp.tile([128, CC, hTB], FR, tag="xnT")
                    for tk in range(hTB // 128):
                        row = ha * hTB + tk * 128
                        xt = lnp.tile([128, C], F32, tag="xt")
                        nc.sync.dma_start(xt, x_in[row:row + 128, :])
                        stats = stp.tile([128, n_sub, 6], F32, tag="st")
                        xt3 = xt.rearrange("p (a b) -> p a b", a=n_sub)
                        for sg in range(n_sub):
                            nc.vector.bn_stats(stats[:, sg, :], xt3[:, sg, :])
                        mv = stp.tile([128, 2], F32, tag="mv")
                        nc.vector.bn_aggr(mv, stats)
                        rstd = stp.tile([128, 1], F32, tag="rstd")
                        nc.scalar.activation(rstd, mv[:, 1:2], AF.Sqrt,
                                             bias=eps_t)
                        nc.vector.reciprocal(rstd, rstd)
                        xn = lnp.tile([128, C], F32, tag="xt", name="xn")
                        nc.vector.tensor_scalar(
                            out=xn, in0=xt, scalar1=mv[:, 0:1], scalar2=rstd,
                            op0=OP.subtract, op1=OP.mult)
                        for cc in range(CC):
                            pt = ps_t.tile([128, 128], F32, tag="pt")
                            nc.tensor.transpose(
                                pt, xn[:, cc * 128:(cc + 1) * 128], ident)
                            nc.vector.tensor_copy(
                                out=xnT[:, cc, tk * 128:(tk + 1) * 128],
                                in_=pt)
                    # Q^T / K^T projections for this token half
                    for (wt_r, dst, bias_sb) in ((wq_r, QT, qb_sb),
                                                 (wk_r, KT, kb_sb)):
                        for p in range(NPAIR):
                            wts = []
                            for cc in range(CC):
                                wt = wqs.tile([128, 128], FR, tag="wqk",
                                              name=f"wqk_{p}_{cc}")
                                nc.sync.dma_start(
                                    wt, _fr(wt_r[:, cc, p * 128:(p + 1) * 128]))
                                wts.append(wt)
                            for tt in range(hTB // PT):
                                ps = ps_p.tile([128, PT], F32, tag="pp")
                                for cc in range(CC):
                                    nc.tensor.matmul(
                                        ps, _fr(wts[cc]),
                                        _fr(xnT[:, cc, tt * PT:(tt + 1) * PT]),
                                        start=(cc == 0), stop=(cc == CC - 1))
                                nc.scalar.activation(
                                    dst[:, p,
                                        ha * hTB + tt * PT:
                                        ha * hTB + (tt + 1) * PT],
                                    ps, AF.Identity, bias=bias_sb[:, p:p + 1])
                    # V in token-major layout, 4 heads per matmul
                    for u in range(NQUAD):
                        wvs = []
                        for cc in range(CC):
                            wt = wqs.tile([128, 256], FR, tag="wv",
                                          name=f"wv_{u}_{cc}")
                            nc.sync.dma_start(
                                wt, _fr(wv_r[:, cc, u * 256:(u + 1) * 256]))
                            wvs.append(wt)
                        for tk in range(hTB // 128):
                            kc = ha * (hTB // 128) + tk
                            ps = ps_p.tile([128, 256], F32, tag="pp", name="pv")
                            for cc in range(CC):
                                nc.tensor.matmul(
                                    ps, _fr(xnT[:, cc, tk * 128:(tk + 1) * 128]),
                                    _fr(wvs[cc]),
                                    start=(cc == 0), stop=(cc == CC - 1))
                            nc.vector.scalar_tensor_tensor(
                                out=V4[:, kc, 4 * u:4 * u + 4, 0:DH],
                                in0=ps.rearrange("p (a b) -> p a b", a=4),
                                scalar=1.0,
                                in1=vb_bc[:, u * 256:(u + 1) * 256]
                                .rearrange("p (a b) -> p a b", a=4),
                                op0=OP.mult, op1=OP.add)

            # ---------------- phase 3+4: attention + Wo + RS ----------------
            dram = top.enter_context(tc.tile_pool(name="dram", bufs=1,
                                                  space="DRAM"))
            partial = dram.tile([TB, C], F32)
            x2c = dram.tile([TOK, C], F32)
            with contextlib.ExitStack() as s:
                wop = s.enter_context(tc.tile_pool(name="wop", bufs=1))
                wo_sb = wop.tile([128, NHC, C], FR)
                nc.sync.dma_start(
                    wo_sb, _fr(wo_in.rearrange("(hc p) n -> p hc n", p=128)))
                ep = s.enter_context(tc.tile_pool(name="ep", bufs=8))
                rcpp = s.enter_context(tc.tile_pool(name="rcpp", bufs=4))
                sbbp = s.enter_context(tc.tile_pool(name="sbbp", bufs=4))
                obp = s.enter_context(tc.tile_pool(name="obp", bufs=4))
                ps_st = s.enter_context(
                    tc.tile_pool(name="ps_st", bufs=3, space="PSUM"))
                ps_av = s.enter_context(
                    tc.tile_pool(name="ps_av", bufs=3, space="PSUM"))
                ps_b = s.enter_context(
                    tc.tile_pool(name="ps_b", bufs=2, space="PSUM"))

                for qt in range(NQT):
                    nkc = (qt + 1) * (QTILE // 128)
                    for h in range(HLOC):
                        p, hi = h // 2, h % 2
                        qsl = QT[64 * hi:64 * hi + 64, p,
                                 qt * QTILE:(qt + 1) * QTILE]
                        pav = ps_av.tile([DH + 1, QTILE], F32, tag="av")
                        for kc in range(nkc):
                            pst = ps_st.tile([128, QTILE], F32, tag="st")
                            nc.tensor.matmul(
                                pst,
                                _fr(KT[64 * hi:64 * hi + 64, p,
                                       kc * 128:(kc + 1) * 128]),
                                _fr(qsl), start=True, stop=True)
                            e = ep.tile([128, QTILE], FR, tag="E")
                            nc.scalar.activation(e, pst, AF.Exp,
                                                 scale=DH ** -0.5)
                            if kc >= nkc - (QTILE // 128):
                                nc.gpsimd.affine_select(
                                    out=e, in_=e, compare_op=OP.is_ge,
                                    fill=0.0,
                                    base=qt * QTILE - kc * 128,
                                    channel_multiplier=-1,
                                    pattern=[[1, QTILE]])
                            nc.tensor.matmul(
                                pav, _fr(V4[:, kc, h, :]), _fr(e),
                                start=(kc == 0), stop=(kc == nkc - 1))
                        rcp = rcpp.tile([1, QTILE], FR, tag="rcp")
                        with nc.allow_low_precision(
                                reason="f32r rounding of softmax recip"):
                            nc.vector.reciprocal(rcp, pav[DH:DH + 1, :])
                        pbc = ps_b.tile([DH, QTILE], F32, tag="bc")
                        nc.tensor.matmul(pbc, _fr(ones1), _fr(rcp),
                                         start=True, stop=True)
                        sbb = sbbp.tile([DH, QTILE], F32, tag="sbb")
                        nc.vector.tensor_copy(out=sbb, in_=pbc)
                        nc.vector.tensor_tensor(
                            out=attnT[64 * hi:64 * hi + 64, p,
                                      qt * QTILE:(qt + 1) * QTILE],
                            in0=pav[0:DH, :], in1=sbb, op=OP.mult)
                    # partial Wo for this qtile, then pairwise reduce-scatter
                    for j2 in range(QTILE // 128):
                        tk = qt * (QTILE // 128) + j2
                        for ct in range(NCT):
                            po = ps_b.tile([128, 512], F32, tag="bc",
                                           name=f"po_{tk}_{ct}")
                            for hc in range(NHC):
                                nc.tensor.matmul(
                                    po,
                                    _fr(attnT[:, hc, tk * 128:(tk + 1) * 128]),
                                    _fr(wo_sb[:, hc, ct * 512:(ct + 1) * 512]),
                                    start=(hc == 0), stop=(hc == NHC - 1))
                            ob = obp.tile([128, 512], F32, tag="ob")
                            nc.vector.tensor_copy(out=ob, in_=po)
                            nc.sync.dma_start(
                                partial[tk * 128:(tk + 1) * 128,
                                        ct * 512:(ct + 1) * 512], ob)
                    if mock_rs:
                        nc.sync.dma_start(
                            x2c[qt * (QTILE // 2):(qt + 1) * (QTILE // 2), :],
                            partial[qt * QTILE:
                                    qt * QTILE + QTILE // 2, :])
                    else:
                        nc.gpsimd.collective_compute(
                            "ReduceScatter", OP.add, replica_groups=groups,
                            ins=[partial[qt * QTILE:(qt + 1) * QTILE, :].opt()],
                            outs=[x2c[qt * (QTILE // 2):(qt + 1) * (QTILE // 2),
                                      :].opt()])

            attn_scope.close()

            # ---------------- phase 5+6: x2 + LN2 + FFN ----------------
            with contextlib.ExitStack() as s:
                cst2 = s.enter_context(tc.tile_pool(name="cst2", bufs=1))
                bo_bc = cst2.tile([128, C], F32)
                nc.sync.dma_start(bo_bc, bcast(bo_in))
                b2_bc = cst2.tile([128, C], F32)
                nc.sync.dma_start(b2_bc, bcast(b2_in))
                b1_sb = cst2.tile([128, NFC], F32)
                nc.sync.dma_start(b1_sb, b1_in)
                x2p = s.enter_context(tc.tile_pool(name="x2p", bufs=2))
                h2tp = s.enter_context(tc.tile_pool(name="h2tp", bufs=2))
                rp = s.enter_context(tc.tile_pool(name="rp", bufs=1))
                lnp2 = s.enter_context(tc.tile_pool(name="lnp2", bufs=4))
                stp2 = s.enter_context(tc.tile_pool(name="stp2", bufs=6))
                w1s = s.enter_context(tc.tile_pool(name="w1s", bufs=3))
                w2s = s.enter_context(tc.tile_pool(name="w2s", bufs=4))
                otp = s.enter_context(tc.tile_pool(name="otp", bufs=4))
                ps_t2 = s.enter_context(
                    tc.tile_pool(name="ps_t2", bufs=2, space="PSUM"))
                ps_u = s.enter_context(
                    tc.tile_pool(name="ps_u", bufs=2, space="PSUM"))
                ps_o = s.enter_context(
                    tc.tile_pool(name="ps_o", bufs=4, space="PSUM"))
                w1_r = w1_in.rearrange("(cc p) f -> p cc f", p=128)

                for hf in range(2):
                    x2sb = x2p.tile([128, NHK, C], F32, tag="x2")
                    h2T = h2tp.tile([128, CC, HALF], FR, tag="h2T")
                    for j in range(NHK):
                        row = hf * HALF + j * 128
                        xq = lnp2.tile([128, C], F32, tag="xq")
                        nc.sync.dma_start(xq, xh_in[row:row + 128, :])
                        rs = lnp2.tile([128, C], F32, tag="xq", name="rs")
                        nc.sync.dma_start(rs, x2c[row:row + 128, :])
                        x2sl = x2sb[:, j, :]
                        nc.vector.tensor_tensor(x2sl, xq, rs, op=OP.add)
                        nc.vector.tensor_tensor(x2sl, x2sl, bo_bc, op=OP.add)
                        stats = stp2.tile([128, n_sub, 6], F32, tag="st2")
                        x23 = x2sl.rearrange("p (a b) -> p a b", a=n_sub)
                        for sg in range(n_sub):
                            nc.vector.bn_stats(stats[:, sg, :], x23[:, sg, :])
                        mv = stp2.tile([128, 2], F32, tag="mv2")
                        nc.vector.bn_aggr(mv, stats)
                        rstd = stp2.tile([128, 1], F32, tag="rstd2")
                        nc.scalar.activation(rstd, mv[:, 1:2], AF.Sqrt,
                                             bias=eps_t)
                        nc.vector.reciprocal(rstd, rstd)
                        xn2 = lnp2.tile([128, C], F32, tag="xq", name="xn2")
                        nc.vector.tensor_scalar(
                            out=xn2, in0=x2sl, scalar1=mv[:, 0:1],
                            scalar2=rstd, op0=OP.subtract, op1=OP.mult)
                        for cc in range(CC):
                            pt = ps_t2.tile([128, 128], F32, tag="pt2")
                            nc.tensor.transpose(
                                pt, xn2[:, cc * 128:(cc + 1) * 128], ident)
                            nc.vector.tensor_copy(
                                out=h2T[:, cc, j * 128:(j + 1) * 128], in_=pt)
                    R = rp.tile([128, NFC, HALF], FR, tag="R")
                    for fc in range(NFC):
                        w1t = w1s.tile([128, CC, 128], FR, tag="w1")
                        nc.sync.dma_start(
                            w1t, _fr(w1_r[:, :, fc * 128:(fc + 1) * 128]))
                        pu = ps_u.tile([128, HALF], F32, tag="pu")
                        for cc in range(CC):
                            nc.tensor.matmul(
                                pu, _fr(w1t[:, cc, :]), _fr(h2T[:, cc, :]),
                                start=(cc == 0), stop=(cc == CC - 1))
                        nc.scalar.activation(R[:, fc, :], pu, AF.Relu,
                                             bias=b1_sb[:, fc:fc + 1])
                    for ct in range(NCT):
                        pos = [ps_o.tile([128, 512], F32, tag="po",
                                         name=f"po_{hf}_{ct}_{tk}")
                               for tk in range(NHK)]
                        for fc in range(NFC):
                            w2t = w2s.tile([128, 512], FR, tag="w2")
                            nc.sync.dma_start(
                                w2t, _fr(w2_in[fc * 128:(fc + 1) * 128,
                                               ct * 512:(ct + 1) * 512]))
                            for tk in range(NHK):
                                nc.tensor.matmul(
                                    pos[tk],
                                    _fr(R[:, fc, tk * 128:(tk + 1) * 128]),
                                    _fr(w2t),
                                    start=(fc == 0), stop=(fc == NFC - 1))
                        for tk in range(NHK):
                            ot = otp.tile([128, 512], F32, tag="ot")
                            nc.vector.scalar_tensor_tensor(
                                out=ot, in0=pos[tk], scalar=1.0,
                                in1=x2sb[:, tk, ct * 512:(ct + 1) * 512],
                                op0=OP.mult, op1=OP.add)
                            nc.vector.tensor_tensor(
                                ot, ot, b2_bc[:, ct * 512:(ct + 1) * 512],
                                op=OP.add)
                            nc.sync.dma_start(
                                out_d[hf * HALF + tk * 128:
                                      hf * HALF + (tk + 1) * 128,
                                      ct * 512:(ct + 1) * 512], ot)
    nc.compile()
    return nc


def prep_inputs(inputs, TB=2048, C=1024, FF=4096, n_devices=8):
    """Fold LN affines into weights; build per-core input maps."""
    f = lambda a: np.ascontiguousarray(np.asarray(a), dtype=np.float32)
    x = f(inputs["x"])
    Wq, Wk, Wv = f(inputs["Wq"]), f(inputs["Wk"]), f(inputs["Wv"])
    Wo, bo = f(inputs["Wo"]), f(inputs["bo"])
    W1, b1 = f(inputs["W1"]), f(inputs["b1"])
    W2, b2 = f(inputs["W2"]), f(inputs["b2"])
    g1, c1 = f(inputs["ln1_g"]), f(inputs["ln1_b"])
    g2, c2 = f(inputs["ln2_g"]), f(inputs["ln2_b"])
    B = x.shape[0]
    NFC = FF // 128
    NPAIR = HLOC // 2

    Wq_f = Wq * g1[None, :, None]   # [H, C, DH]
    Wk_f = Wk * g1[None, :, None]
    Wv_f = Wv * g1[None, :, None]
    qb_f = np.einsum("c,hcd->hd", c1, Wq)
    kb_f = np.einsum("c,hcd->hd", c1, Wk)
    vb_f = np.einsum("c,hcd->hd", c1, Wv)
    W1_f = g2[:, None] * W1
    b1_f = b1 + c2 @ W1

    in_maps = []
    for c in range(n_devices):
        b, g = c // 2, c % 2
        hsl = slice(HLOC * g, HLOC * (g + 1))
        wq_c = np.ascontiguousarray(
            Wq_f[hsl].transpose(1, 0, 2).reshape(C, HLOC * DH))
        wk_c = np.ascontiguousarray(
            Wk_f[hsl].transpose(1, 0, 2).reshape(C, HLOC * DH))
        wv_c = np.ascontiguousarray(
            Wv_f[hsl].transpose(1, 0, 2).reshape(C, HLOC * DH))
        qb_c = np.ascontiguousarray(
            qb_f[hsl].reshape(NPAIR, 128).T)
        kb_c = np.ascontiguousarray(
            kb_f[hsl].reshape(NPAIR, 128).T)
        vb_c = np.ascontiguousarray(vb_f[hsl].reshape(HLOC * DH))
        wo_c = np.ascontiguousarray(Wo[HLOC * DH * g:HLOC * DH * (g + 1)])
        xh_c = np.ascontiguousarray(
            x[b].reshape(TB // QTILE, 2, QTILE // 2, C)[:, g]
            .reshape(TB // 2, C))
        in_maps.append({
            "x": np.ascontiguousarray(x[b]),
            "xh": xh_c,
            "wq": wq_c, "wk": wk_c, "wv": wv_c,
            "qb": qb_c, "kb": kb_c, "vb": vb_c,
            "wo": wo_c, "bo": bo,
            "w1": np.ascontiguousarray(W1_f),
            "b1f": np.ascontiguousarray(b1_f.reshape(NFC, 128).T),
            "w2": W2, "b2": b2,
        })
    return in_maps


def assemble_output(results, x_shape, TB=2048, C=1024):
    B = x_shape[0]
    out = np.empty((B, TB, C), dtype=np.float32)
    for c, r in enumerate(results):
        b, g = c // 2, c % 2
        out[b].reshape(TB // QTILE, 2, QTILE // 2, C)[:, g] = \
            r["out"].reshape(TB // QTILE, QTILE // 2, C)
    return out


_NC_CACHE = {}


def _get_nc(TB=2048, C=1024, FF=4096):
    key = (TB, C, FF)
    if key not in _NC_CACHE:
        _NC_CACHE[key] = build_decoder_nc(TB, C, FF)
    return _NC_CACHE[key]


def run_hw(inputs, trace=False, trace_kwargs=None):
    from concourse.bass_utils import run_bass_kernel_spmd
    x = np.asarray(inputs["x"])
    B, TB, C = x.shape
    FF = np.asarray(inputs["W1"]).shape[1]
    nc = _get_nc(TB, C, FF)
    in_maps = prep_inputs(inputs, TB, C, FF)
    res = run_bass_kernel_spmd(nc, in_maps, core_ids=list(range(8)),
                               trace=trace, **(trace_kwargs or {}))
    out = assemble_output(res.results, x.shape, TB, C)
    return out, res


def kernel(**inputs):
    out, _ = run_hw(inputs, trace=False)
    return out


# revision 19
# speedup vs baseline: 1.2705x; 1.2705x over previous
"""Trainium2 Bass kernel for a 16-head causal decoder block.

Sharding: 8 cores = 4 batches x 2 head-groups (8 heads each).
Per core: LN1 -> Q/K/V proj (its heads) -> causal attention -> partial Wo
-> pairwise ReduceScatter of x2 partials (chunked per 512-token qtile)
-> token-sharded LN2 + FFN (1024 tokens per core) -> output slice.

All matmuls run as float32r (FP22 truncation, full PE rate at N>=256).
LayerNorm affines are folded into the projection weights on the host.
"""

import os
import sys

import numpy as np


def _ensure_path():
    try:
        import concourse.bass  # noqa: F401
        return
    except ImportError:
        pass
    for p in ("/opt/trn_rl_repo", "/root/.axon_site/_ro/trn_rl_repo"):
        if os.path.isdir(p) and p not in sys.path:
            sys.path.insert(0, p)
    import concourse.bass  # noqa: F401


_ensure_path()

import concourse.bass as bass  # noqa: E402
import concourse.mybir as mybir  # noqa: E402
import concourse.tile as tile  # noqa: E402
from concourse import bacc  # noqa: E402
from concourse.masks import make_identity  # noqa: E402

F32 = mybir.dt.float32
FR = mybir.dt.float32r
AF = mybir.ActivationFunctionType
OP = mybir.AluOpType

DH = 64          # head dim
HLOC = 8         # heads per core
QTILE = 512      # query tile
EPS = 1e-5


def _fr(ap):
    return ap.bitcast(FR)


def build_decoder_nc(TB=2048, C=1024, FF=4096, n_devices=8, mock_rs=False):
    """Build the SPMD Bass program. TB: tokens/batch, C: embed, FF: ff dim."""
    assert TB % QTILE == 0 and C % 512 == 0 and FF % 128 == 0
    CC = C // 128          # C chunks
    NQT = TB // QTILE      # query tiles
    NKC = TB // 128        # key chunks
    NFC = FF // 128        # ff chunks
    NCT = C // 512         # output C tiles
    TOK = TB // 2          # output tokens per core
    HALF = TOK // 2        # ffn token half
    NHK = HALF // 128      # token chunks per ffn half
    NPAIR = HLOC // 2
    NQUAD = HLOC // 4
    NHC = HLOC * DH // 128  # attn-out hd chunks (4)
    groups = [[2 * i, 2 * i + 1] for i in range(n_devices // 2)]

    nc = bacc.Bacc("TRN2", target_bir_lowering=False, debug=False,
                   num_devices=n_devices)

    x_in = nc.dram_tensor("x", [TB, C], F32, kind="ExternalInput").ap()
    xh_in = nc.dram_tensor("xh", [TOK, C], F32, kind="ExternalInput").ap()
    wq_in = nc.dram_tensor("wq", [C, HLOC * DH], F32, kind="ExternalInput").ap()
    wk_in = nc.dram_tensor("wk", [C, HLOC * DH], F32, kind="ExternalInput").ap()
    wv_in = nc.dram_tensor("wv", [C, HLOC * DH], F32, kind="ExternalInput").ap()
    qb_in = nc.dram_tensor("qb", [128, NPAIR], F32, kind="ExternalInput").ap()
    kb_in = nc.dram_tensor("kb", [128, NPAIR], F32, kind="ExternalInput").ap()
    vb_in = nc.dram_tensor("vb", [HLOC * DH], F32, kind="ExternalInput").ap()
    wo_in = nc.dram_tensor("wo", [HLOC * DH, C], F32, kind="ExternalInput").ap()
    bo_in = nc.dram_tensor("bo", [C], F32, kind="ExternalInput").ap()
    w1_in = nc.dram_tensor("w1", [C, FF], F32, kind="ExternalInput").ap()
    b1_in = nc.dram_tensor("b1f", [128, NFC], F32, kind="ExternalInput").ap()
    w2_in = nc.dram_tensor("w2", [FF, C], F32, kind="ExternalInput").ap()
    b2_in = nc.dram_tensor("b2", [C], F32, kind="ExternalInput").ap()
    out_d = nc.dram_tensor("out", [TOK, C], F32, kind="ExternalOutput").ap()

    def bcast(ap1d, p=128):
        return bass.AP(tensor=ap1d.tensor, offset=ap1d.offset,
                       ap=[[0, p]] + [list(d) for d in ap1d.ap])

    n_sub = max(1, C // 512)

    NG = TB // QTILE  # 512-token groups; group g == qtile g

    with tile.TileContext(nc) as tc:
        import contextlib
        with contextlib.ExitStack() as top:
            const = top.enter_context(tc.tile_pool(name="const", bufs=1))
            ident = const.tile([128, 128], F32)
            make_identity(nc, ident)
            ones1 = const.tile([1, DH], FR)
            ones1f = const.tile([1, DH], F32)
            nc.vector.memset(ones1f, 1.0)
            nc.scalar.copy(ones1, ones1f)
            eps_t = const.tile([128, 1], F32)
            nc.vector.memset(eps_t, EPS)

            dram = top.enter_context(tc.tile_pool(name="dram", bufs=1,
                                                  space="DRAM"))
            partial = dram.tile([TB, C], F32)
            x2c = dram.tile([TOK, C], F32)

            # ---- phases 1-4 interleaved per 512-token group ----
            with contextlib.ExitStack() as s:
                cst1 = s.enter_context(tc.tile_pool(name="cst1", bufs=1))
                qb_sb = cst1.tile([128, NPAIR], F32)
                nc.sync.dma_start(qb_sb, qb_in)
                kb_sb = cst1.tile([128, NPAIR], F32)
                nc.sync.dma_start(kb_sb, kb_in)
                vb_bc = cst1.tile([128, HLOC * DH], F32)
                nc.sync.dma_start(vb_bc, bcast(vb_in))
                onesv = cst1.tile([128, NKC * HLOC], F32)
                nc.vector.memset(onesv, 1.0)
                wo_sb = cst1.tile([128, NHC, C], FR)

                kv = s.enter_context(tc.tile_pool(name="kv", bufs=1))
                KT = kv.tile([128, NPAIR, TB], FR)      # K^T, head pairs
                V4 = kv.tile([128, NKC, HLOC, DH + 1], FR)  # V + ones col
                nc.vector.tensor_copy(
                    out=V4[:, :, :, DH],
                    in_=onesv.rearrange("p (a b) -> p a b", a=NKC))

                qtp = s.enter_context(tc.tile_pool(name="qtp", bufs=2))
                atp = s.enter_context(tc.tile_pool(name="atp", bufs=2))
                wqs = s.enter_context(tc.tile_pool(name="wqs", bufs=10))
                lnp = s.enter_context(tc.tile_pool(name="lnp", bufs=4))
                stp = s.enter_context(tc.tile_pool(name="stp", bufs=6))
                xntp = s.enter_context(tc.tile_pool(name="xntp", bufs=2))
                ep = s.enter_context(tc.tile_pool(name="ep", bufs=6))
                rcpp = s.enter_context(tc.tile_pool(name="rcpp", bufs=2))
                sbbp = s.enter_context(tc.tile_pool(name="sbbp", bufs=2))
                obp = s.enter_context(tc.tile_pool(name="obp", bufs=2))
                ps_t = s.enter_context(
                    tc.tile_pool(name="ps_t", bufs=2, space="PSUM"))
                ps_p = s.enter_context(
                    tc.tile_pool(name="ps_p", bufs=2, space="PSUM"))
                ps_st = s.enter_context(
                    tc.tile_pool(name="ps_st", bufs=2, space="PSUM"))
                ps_av = s.enter_context(
                    tc.tile_pool(name="ps_av", bufs=2, space="PSUM"))
                wq_r = wq_in.rearrange("(cc p) n -> p cc n", p=128)
                wk_r = wk_in.rearrange("(cc p) n -> p cc n", p=128)
                wv_r = wv_in.rearrange("(cc p) n -> p cc n", p=128)

                for g in range(NG):
                    if g == 0:
                        nc.sync.dma_start(
                            wo_sb,
                            _fr(wo_in.rearrange("(hc p) n -> p hc n", p=128)))
                    # --- LN1 for this group -> xnT (C-major) ---
                    xnT = xntp.tile([128, CC, QTILE], FR, tag="xnT",
                                    name=f"xnT_{g}")
                    for tk in range(QTILE // 128):
                        row = g * QTILE + tk * 128
                        xt = lnp.tile([128, C], F32, tag="xt",
                                      name=f"xt_{g}_{tk}")
                        nc.sync.dma_start(xt, x_in[row:row + 128, :])
                        stats = stp.tile([128, n_sub, 6], F32, tag="st")
                        xt3 = xt.rearrange("p (a b) -> p a b", a=n_sub)
                        for sg in range(n_sub):
                            nc.vector.bn_stats(stats[:, sg, :], xt3[:, sg, :])
                        mv = stp.tile([128, 2], F32, tag="mv")
                        nc.vector.bn_aggr(mv, stats)
                        rstd = stp.tile([128, 1], F32, tag="rstd")
                        nc.scalar.activation(rstd, mv[:, 1:2], AF.Sqrt,
                                             bias=eps_t)
                        nc.vector.reciprocal(rstd, rstd)
                        xn = lnp.tile([128, C], F32, tag="xt",
                                      name=f"xn_{g}_{tk}")
                        nc.vector.tensor_scalar(
                            out=xn, in0=xt, scalar1=mv[:, 0:1], scalar2=rstd,
                            op0=OP.subtract, op1=OP.mult)
                        for cc in range(CC):
                            pt = ps_t.tile([128, 128], F32, tag="pt")
                            nc.tensor.transpose(
                                pt, xn[:, cc * 128:(cc + 1) * 128], ident)
                            nc.vector.tensor_copy(
                                out=xnT[:, cc, tk * 128:(tk + 1) * 128],
                                in_=pt)
                    # --- Q^T/K^T/V projections for this group ---
                    QTg = qtp.tile([128, NPAIR, QTILE], FR, tag="QT",
                                   name=f"QT_{g}")
                    for (wt_r, dst, bias_sb, isq) in ((wq_r, QTg, qb_sb, 1),
                                                      (wk_r, KT, kb_sb, 0)):
                        for p in range(NPAIR):
                            wts = []
                            for cc in range(CC):
                                wt = wqs.tile([128, 128], FR, tag="wqk",
                                              name=f"wqk_{g}_{p}_{cc}")
                                nc.sync.dma_start(
                                    wt, _fr(wt_r[:, cc, p * 128:(p + 1) * 128]))
                                wts.append(wt)
                            ps = ps_p.tile([128, QTILE], F32, tag="pp")
                            for cc in range(CC):
                                nc.tensor.matmul(
                                    ps, wts[cc],
                                    xnT[:, cc, :],
                                    start=(cc == 0), stop=(cc == CC - 1))
                            if isq:
                                nc.scalar.activation(
                                    QTg[:, p, :], ps, AF.Identity,
                                    bias=bias_sb[:, p:p + 1])
                            else:
                                nc.scalar.activation(
                                    KT[:, p, g * QTILE:(g + 1) * QTILE],
                                    ps, AF.Identity,
                                    bias=bias_sb[:, p:p + 1])
                    for u in range(NQUAD):
                        wvs = []
                        for cc in range(CC):
                            wt = wqs.tile([128, 256], FR, tag="wv",
                                          name=f"wv_{g}_{u}_{cc}")
                            nc.sync.dma_start(
                                wt, _fr(wv_r[:, cc, u * 256:(u + 1) * 256]))
                            wvs.append(wt)
                        for tk in range(QTILE // 128):
                            kc = g * (QTILE // 128) + tk
                            ps = ps_p.tile([128, 256], F32, tag="pp",
                                           name="pv")
                            for cc in range(CC):
                                nc.tensor.matmul(
                                    ps, xnT[:, cc, tk * 128:(tk + 1) * 128],
                                    wvs[cc],
                                    start=(cc == 0), stop=(cc == CC - 1))
                            nc.vector.scalar_tensor_tensor(
                                out=V4[:, kc, 4 * u:4 * u + 4, 0:DH],
                                in0=ps.rearrange("p (a b) -> p a b", a=4),
                                scalar=1.0,
                                in1=vb_bc[:, u * 256:(u + 1) * 256]
                                .rearrange("p (a b) -> p a b", a=4),
                                op0=OP.mult, op1=OP.add)
                    # --- attention for qtile g (keys 0 .. (g+1)*512) ---
                    attnTg = atp.tile([128, NHC, QTILE], FR, tag="attnT",
                                      name=f"attnT_{g}")
                    nkc = (g + 1) * (QTILE // 128)
                    for h in range(HLOC):
                        p, hi = h // 2, h % 2
                        qsl = QTg[64 * hi:64 * hi + 64, p, :]
                        pav = ps_av.tile([DH + 1, QTILE], F32, tag="av")
                        for kc in range(nkc):
                            pst = ps_st.tile([128, QTILE], F32, tag="st")
                            nc.tensor.matmul(
                                pst,
                                KT[64 * hi:64 * hi + 64, p,
                                   kc * 128:(kc + 1) * 128],
                                qsl, start=True, stop=True)
                            e = ep.tile([128, QTILE], FR, tag="E")
                            nc.scalar.activation(e, pst, AF.Exp,
                                                 scale=DH ** -0.5)
                            if kc >= nkc - (QTILE // 128):
                                nc.gpsimd.affine_select(
                                    out=e, in_=e, compare_op=OP.is_ge,
                                    fill=0.0,
                                    base=g * QTILE - kc * 128,
                                    channel_multiplier=-1,
                                    pattern=[[1, QTILE]])
                            nc.tensor.matmul(
                                pav, V4[:, kc, h, :], e,
                                start=(kc == 0), stop=(kc == nkc - 1))
                        rcp = rcpp.tile([1, QTILE], FR, tag="rcp")
                        with nc.allow_low_precision(
                                reason="f32r rounding of softmax recip"):
                            nc.vector.reciprocal(rcp, pav[DH:DH + 1, :])
                        pbc = ps_p.tile([DH, QTILE], F32, tag="pp",
                                        name="pbc")
                        nc.tensor.matmul(pbc, ones1, rcp,
                                         start=True, stop=True)
                        sbb = sbbp.tile([DH, QTILE], F32, tag="sbb")
                        nc.vector.tensor_copy(out=sbb, in_=pbc)
                        nc.vector.tensor_tensor(
                            out=attnTg[64 * hi:64 * hi + 64, p, :],
                            in0=pav[0:DH, :], in1=sbb, op=OP.mult)
                    # --- partial Wo for qtile g, then pairwise RS ---
                    for j2 in range(QTILE // 128):
                        tk = g * (QTILE // 128) + j2
                        for ct in range(NCT):
                            po = ps_p.tile([128, 512], F32, tag="pp",
                                           name=f"po_{tk}_{ct}")
                            for hc in range(NHC):
                                nc.tensor.matmul(
                                    po,
                                    attnTg[:, hc, j2 * 128:(j2 + 1) * 128],
                                    wo_sb[:, hc, ct * 512:(ct + 1) * 512],
                                    start=(hc == 0), stop=(hc == NHC - 1))
                            ob = obp.tile([128, 512], F32, tag="ob")
                            nc.vector.tensor_copy(out=ob, in_=po)
                            nc.sync.dma_start(
                                partial[tk * 128:(tk + 1) * 128,
                                        ct * 512:(ct + 1) * 512], ob)
                    if mock_rs:
                        nc.sync.dma_start(
                            x2c[g * (QTILE // 2):(g + 1) * (QTILE // 2), :],
                            partial[g * QTILE:
                                    g * QTILE + QTILE // 2, :])
                    else:
                        nc.gpsimd.collective_compute(
                            "ReduceScatter", OP.add, replica_groups=groups,
                            ins=[partial[g * QTILE:(g + 1) * QTILE, :].opt()],
                            outs=[x2c[g * (QTILE // 2):
                                      (g + 1) * (QTILE // 2), :].opt()])

            # ---------------- phase 5+6: x2 + LN2 + FFN ----------------
            with contextlib.ExitStack() as s:
                cst2 = s.enter_context(tc.tile_pool(name="cst2", bufs=1))
                bo_bc = cst2.tile([128, C], F32)
                nc.sync.dma_start(bo_bc, bcast(bo_in))
                b2_bc = cst2.tile([128, C], F32)
                nc.sync.dma_start(b2_bc, bcast(b2_in))
                b1_sb = cst2.tile([128, NFC], F32)
                nc.sync.dma_start(b1_sb, b1_in)
                x2p = s.enter_context(tc.tile_pool(name="x2p", bufs=2))
                h2tp = s.enter_context(tc.tile_pool(name="h2tp", bufs=2))
                rp = s.enter_context(tc.tile_pool(name="rp", bufs=1))
                lnp2 = s.enter_context(tc.tile_pool(name="lnp2", bufs=4))
                stp2 = s.enter_context(tc.tile_pool(name="stp2", bufs=6))
                w1s = s.enter_context(tc.tile_pool(name="w1s", bufs=3))
                w2s = s.enter_context(tc.tile_pool(name="w2s", bufs=4))
                otp = s.enter_context(tc.tile_pool(name="otp", bufs=4))
                ps_t2 = s.enter_context(
                    tc.tile_pool(name="ps_t2", bufs=2, space="PSUM"))
                ps_u = s.enter_context(
                    tc.tile_pool(name="ps_u", bufs=2, space="PSUM"))
                ps_o = s.enter_context(
                    tc.tile_pool(name="ps_o", bufs=4, space="PSUM"))
                w1_r = w1_in.rearrange("(cc p) f -> p cc f", p=128)

                for hf in range(2):
                    x2sb = x2p.tile([128, NHK, C], F32, tag="x2")
                    h2T = h2tp.tile([128, CC, HALF], FR, tag="h2T")
                    for j in range(NHK):
                        row = hf * HALF + j * 128
                        xq = lnp2.tile([128, C], F32, tag="xq")
                        nc.sync.dma_start(xq, xh_in[row:row + 128, :])
                        rs = lnp2.tile([128, C], F32, tag="xq", name="rs")
                        nc.sync.dma_start(rs, x2c[row:row + 128, :])
                        x2sl = x2sb[:, j, :]
                        nc.vector.tensor_tensor(x2sl, xq, rs, op=OP.add)
                        nc.vector.tensor_tensor(x2sl, x2sl, bo_bc, op=OP.add)
                        stats = stp2.tile([128, n_sub, 6], F32, tag="st2")
                        x23 = x2sl.rearrange("p (a b) -> p a b", a=n_sub)
                        for sg in range(n_sub):
                            nc.vector.bn_stats(stats[:, sg, :], x23[:, sg, :])
                        mv = stp2.tile([128, 2], F32, tag="mv2")
                        nc.vector.bn_aggr(mv, stats)
                        rstd = stp2.tile([128, 1], F32, tag="rstd2")
                        nc.scalar.activation(rstd, mv[:, 1:2], AF.Sqrt,
                                             bias=eps_t)
                        nc.vector.reciprocal(rstd, rstd)
                        xn2 = lnp2.tile([128, C], F32, tag="xq", name="xn2")
                        nc.vector.tensor_scalar(
                            out=xn2, in0=x2sl, scalar1=mv[:, 0:1],
                            scalar2=rstd, op0=OP.subtract, op1=OP.mult)
                        for cc in range(CC):
                            pt = ps_t2.tile([128, 128], F32, tag="pt2")
                            nc.tensor.transpose(
                                pt, xn2[:, cc * 128:(cc + 1) * 128], ident)
                            nc.vector.tensor_copy(
                                out=h2T[:, cc, j * 128:(j + 1) * 128], in_=pt)
                    R = rp.tile([128, NFC, HALF], FR, tag="R")
                    poss = {}
                    poss[0] = [ps_o.tile([128, 512], F32, tag="po",
                                         name=f"po_{hf}_0_{tk}")
                               for tk in range(NHK)]
                    for fc in range(NFC):
                        w1t = w1s.tile([128, CC, 128], FR, tag="w1")
                        nc.sync.dma_start(
                            w1t, _fr(w1_r[:, :, fc * 128:(fc + 1) * 128]))
                        pu = ps_u.tile([128, HALF], F32, tag="pu")
                        for cc in range(CC):
                            nc.tensor.matmul(
                                pu, w1t[:, cc, :], h2T[:, cc, :],
                                start=(cc == 0), stop=(cc == CC - 1))
                        nc.scalar.activation(R[:, fc, :], pu, AF.Relu,
                                             bias=b1_sb[:, fc:fc + 1])
                        # W2 ct=0 consumes R[fc] immediately
                        w2t = w2s.tile([128, 512], FR, tag="w2",
                                       name=f"w2_{hf}_0_{fc}")
                        nc.sync.dma_start(
                            w2t, _fr(w2_in[fc * 128:(fc + 1) * 128, 0:512]))
                        for tk in range(NHK):
                            nc.tensor.matmul(
                                poss[0][tk],
                                R[:, fc, tk * 128:(tk + 1) * 128], w2t,
                                start=(fc == 0), stop=(fc == NFC - 1))
                    for ct in range(NCT):
                        if ct > 0:
                            poss[ct] = [ps_o.tile([128, 512], F32, tag="po",
                                                  name=f"po_{hf}_{ct}_{tk}")
                                        for tk in range(NHK)]
                            for fc in range(NFC):
                                w2t = w2s.tile([128, 512], FR, tag="w2",
                                               name=f"w2_{hf}_{ct}_{fc}")
                                nc.sync.dma_start(
                                    w2t,
                                    _fr(w2_in[fc * 128:(fc + 1) * 128,
                                              ct * 512:(ct + 1) * 512]))
                                for tk in range(NHK):
                                    nc.tensor.matmul(
                                        poss[ct][tk],
                                        R[:, fc, tk * 128:(tk + 1) * 128],
                                        w2t,
                                        start=(fc == 0), stop=(fc == NFC - 1))
                        for tk in range(NHK):
                            ot = otp.tile([128, 512], F32, tag="ot")
                            nc.vector.scalar_tensor_tensor(
                                out=ot, in0=poss[ct][tk], scalar=1.0,
                                in1=x2sb[:, tk, ct * 512:(ct + 1) * 512],
                                op0=OP.mult, op1=OP.add)
                            nc.vector.tensor_tensor(
                                ot, ot, b2_bc[:, ct * 512:(ct + 1) * 512],
                                op=OP.add)
                            nc.sync.dma_start(
                                out_d[hf * HALF + tk * 128:
                                      hf * HALF + (tk + 1) * 128,
                                      ct * 512:(ct + 1) * 512], ot)
    nc.compile()
    return nc


def prep_inputs(inputs, TB=2048, C=1024, FF=4096, n_devices=8):
    """Fold LN affines into weights; build per-core input maps."""
    f = lambda a: np.ascontiguousarray(np.asarray(a), dtype=np.float32)
    x = f(inputs["x"])
    Wq, Wk, Wv = f(inputs["Wq"]), f(inputs["Wk"]), f(inputs["Wv"])
    Wo, bo = f(inputs["Wo"]), f(inputs["bo"])
    W1, b1 = f(inputs["W1"]), f(inputs["b1"])
    W2, b2 = f(inputs["W2"]), f(inputs["b2"])
    g1, c1 = f(inputs["ln1_g"]), f(inputs["ln1_b"])
    g2, c2 = f(inputs["ln2_g"]), f(inputs["ln2_b"])
    B = x.shape[0]
    NFC = FF // 128
    NPAIR = HLOC // 2

    Wq_f = Wq * g1[None, :, None]   # [H, C, DH]
    Wk_f = Wk * g1[None, :, None]
    Wv_f = Wv * g1[None, :, None]
    qb_f = np.einsum("c,hcd->hd", c1, Wq)
    kb_f = np.einsum("c,hcd->hd", c1, Wk)
    vb_f = np.einsum("c,hcd->hd", c1, Wv)
    W1_f = g2[:, None] * W1
    b1_f = b1 + c2 @ W1

    in_maps = []
    for c in range(n_devices):
        b, g = c // 2, c % 2
        hsl = slice(HLOC * g, HLOC * (g + 1))
        wq_c = np.ascontiguousarray(
            Wq_f[hsl].transpose(1, 0, 2).reshape(C, HLOC * DH))
        wk_c = np.ascontiguousarray(
            Wk_f[hsl].transpose(1, 0, 2).reshape(C, HLOC * DH))
        wv_c = np.ascontiguousarray(
            Wv_f[hsl].transpose(1, 0, 2).reshape(C, HLOC * DH))
        qb_c = np.ascontiguousarray(
            qb_f[hsl].reshape(NPAIR, 128).T)
        kb_c = np.ascontiguousarray(
            kb_f[hsl].reshape(NPAIR, 128).T)
        vb_c = np.ascontiguousarray(vb_f[hsl].reshape(HLOC * DH))
        wo_c = np.ascontiguousarray(Wo[HLOC * DH * g:HLOC * DH * (g + 1)])
        xh_c = np.ascontiguousarray(
            x[b].reshape(TB // QTILE, 2, QTILE // 2, C)[:, g]
            .reshape(TB // 2, C))
        in_maps.append({
            "x": np.ascontiguousarray(x[b]),
            "xh": xh_c,
            "wq": wq_c, "wk": wk_c, "wv": wv_c,
            "qb": qb_c, "kb": kb_c, "vb": vb_c,
            "wo": wo_c, "bo": bo,
            "w1": np.ascontiguousarray(W1_f),
            "b1f": np.ascontiguousarray(b1_f.reshape(NFC, 128).T),
            "w2": W2, "b2": b2,
        })
    return in_maps


def assemble_output(results, x_shape, TB=2048, C=1024):
    B = x_shape[0]
    out = np.empty((B, TB, C), dtype=np.float32)
    for c, r in enumerate(results):
        b, g = c // 2, c % 2
        out[b].reshape(TB // QTILE, 2, QTILE // 2, C)[:, g] = \
            r["out"].reshape(TB // QTILE, QTILE // 2, C)
    return out


_NC_CACHE = {}


def _get_nc(TB=2048, C=1024, FF=4096):
    key = (TB, C, FF)
    if key not in _NC_CACHE:
        _NC_CACHE[key] = build_decoder_nc(TB, C, FF)
    return _NC_CACHE[key]


def run_hw(inputs, trace=False, trace_kwargs=None):
    from concourse.bass_utils import run_bass_kernel_spmd
    x = np.asarray(inputs["x"])
    B, TB, C = x.shape
    FF = np.asarray(inputs["W1"]).shape[1]
    nc = _get_nc(TB, C, FF)
    in_maps = prep_inputs(inputs, TB, C, FF)
    res = run_bass_kernel_spmd(nc, in_maps, core_ids=list(range(8)),
                               trace=trace, **(trace_kwargs or {}))
    out = assemble_output(res.results, x.shape, TB, C)
    return out, res


def kernel(**inputs):
    out, _ = run_hw(inputs, trace=False)
    return out
